# revision 20
# baseline (speedup 1.0000x reference)
"""Causal attention (B=2, T=2048, E=1024, H=16, D=64) on 8 TRN2 NeuronCores.

Sharding: core c handles batch b = c//4 and local head group hg = c%4
(4 heads, 256 head-dims).  Data parallel over batch, tensor parallel over
heads; the output projection is row-parallel, so each core returns a
partial [T, E] output and the host sums the 4 partials per batch (bias
is pre-divided by 4 and added on-device).

Device plan (per core, all-bf16 matmuls with fp32 PSUM accumulation):
  xt  = x[b].T                   [E, T]  (host-transposed; e on partitions)
  wqt/wkt/wvt = W[h].T           pre-tiled [P, 8, 256] for SBUF layout
  wpt = Wp[:, h].T               pre-tiled [P, 2, 1024]

Schedule: TRN2's PE p-state ramps (0.65->1.2->2.4 GHz over ~3us of
CONTINUOUS work; any idle gap resets it), so the whole kernel is built
as one unbroken PE instruction stream:
  - xt is DMAed in 32 [128,512] chunks in tb-major (consumption) order
    on the sync ring; wq/wk (in halves) + wv stream on the scalar ring.
    First matmul starts as soon as wq-half0 + xt(ec0,tb0) land.
  - pre-attention: q/k for token-block tb0 (both head pairs) + v(t0-3)
    in an 8-bank PSUM pool, ec-inner so matmuls chase chunk arrivals.
  - attention(hp, ib) runs as soon as its q/k/v deps exist; ALL other
    work (q/k tb1-3, v t4-15, output projection) is emitted as ~1-2us
    filler closures between attention pipeline steps, sized so the PE
    never idles while ScalarE's exp drains (j-loop software-pipelined
    one step: scores/exp for jb+1 are emitted before the PV matmuls of
    jb).
  - scores st[j, i] = q_j . k_i with 2-head row-packing (two K=64
    matmuls in distinct PE row groups), exp on ScalarE (scale=1/8; no
    max subtraction -- scores are ~N(0,1) so exp cannot overflow),
    causal mask multiply only on block-diagonal tiles, PV accumulation
    over j in PSUM with 64 ones-columns prepended to v (softmax
    denominator comes out of the PV matmul on partitions 0:63), then
    approx-reciprocal + multiply for the normalization.
  - v ones-columns are written by GpSimd (otherwise idle) to keep
    VectorE off the critical path.
"""

import ml_dtypes
import numpy as np

import concourse.bass as bass
import concourse.tile as tile
from concourse import bacc, mybir
from concourse.bass_utils import run_bass_kernel_spmd

B, T, E = 2, 2048, 1024
H, D = 16, 64
NCORES = 8
GROUPS = 4              # cores per batch (tensor parallel over heads)
HL = H // GROUPS        # 4 local heads per core
HDL = HL * D            # 256 local head dims
P = 128
TQ = 512                # i-block (free dim of score tiles)
JB = 128                # j-block (partition dim of score tiles)
N_TB = T // TQ          # 4
N_EC = E // P           # 8
N_TC = T // P           # 16

F32 = mybir.dt.float32
BF16 = mybir.dt.bfloat16
AF = mybir.ActivationFunctionType


def _build_nc():
    nc = bacc.Bacc("TRN2", target_bir_lowering=False, debug=False)
    xt = nc.dram_tensor("xt", [E, T], BF16, kind="ExternalInput").ap()
    wqt = nc.dram_tensor("wqt", [P, N_EC, HDL], BF16, kind="ExternalInput").ap()
    wkt = nc.dram_tensor("wkt", [P, N_EC, HDL], BF16, kind="ExternalInput").ap()
    wvt = nc.dram_tensor("wvt", [P, N_EC, HDL], BF16, kind="ExternalInput").ap()
    wpt = nc.dram_tensor("wpt", [P, 2, E], BF16, kind="ExternalInput").ap()
    bqv = nc.dram_tensor("bqv", [HDL], F32, kind="ExternalInput").ap()
    bkv = nc.dram_tensor("bkv", [HDL], F32, kind="ExternalInput").ap()
    bvv = nc.dram_tensor("bvv", [HDL], F32, kind="ExternalInput").ap()
    bp4 = nc.dram_tensor("bp4", [E], F32, kind="ExternalInput").ap()
    maskd = nc.dram_tensor("mask", [GROUPS, JB, TQ], BF16,
                           kind="ExternalInput").ap()
    onesv = nc.dram_tensor("onesv", [HDL], BF16, kind="ExternalInput").ap()
    out = nc.dram_tensor("out", [T, E], BF16, kind="ExternalOutput").ap()

    with tile.TileContext(nc) as tc:
        with (
            tc.tile_pool(name="big", bufs=1) as big,
            tc.tile_pool(name="work", bufs=5) as work,
            tc.tile_pool(name="outp", bufs=3) as outp,
        ):
            # ---------------- input loads ---------------------------------
            # DMA rings move contiguous 512KB descriptors at ~330GB/s but
            # strided gathers collapse to ~35GB/s, so xt ships as 8 full-T
            # contiguous e-chunks alternating between the two HWDGE rings;
            # weights lead the scalar ring, wv rides the sync ring between
            # xt chunks.  Expected arrivals (ring start ~6.7us, 1.55us per
            # 512KB): sync e0 8.3, e2 9.9, wv 11.4, e4 13, e6 14.6;
            # scalar wq 8.3, wk 9.9, e1 11.4, e3 13, e5 14.6, e7 16.1.
            xt_sb = [big.tile([P, T], BF16, tag=f"xt{ec}", name=f"xt{ec}")
                     for ec in range(N_EC)]
            wq_all = big.tile([P, N_EC, HDL], BF16, tag="wq", name="wq")
            wk_all = big.tile([P, N_EC, HDL], BF16, tag="wk", name="wk")
            wv_all = big.tile([P, N_EC, HDL], BF16, tag="wv", name="wv")

            def xt_dma(eng, ec):
                eng.dma_start(xt_sb[ec], xt[ec * P:(ec + 1) * P, :])

            xt_dma(nc.sync, 0)
            nc.scalar.dma_start(wq_all, wqt)
            xt_dma(nc.sync, 2)
            nc.scalar.dma_start(wk_all, wkt)
            nc.sync.dma_start(wv_all, wvt)
            xt_dma(nc.scalar, 1)
            xt_dma(nc.sync, 4)
            xt_dma(nc.scalar, 3)
            xt_dma(nc.sync, 6)
            xt_dma(nc.scalar, 5)
            xt_dma(nc.scalar, 7)

            def xts(ec, tb):
                return xt_sb[ec][:, tb * TQ:(tb + 1) * TQ]
            # PE warm-up scratch: matmuls on this (memset) tile run while
            # the first input DMAs are in flight, so the PE p-state ramp
            # (0.65->2.4GHz over ~3us of continuous work) completes before
            # real data arrives instead of slowing the first real matmuls.
            warm_sb = big.tile([P, HDL], BF16, tag="warm", name="warm")
            nc.gpsimd.memset(warm_sb, 0.0)
            # gpsimd (SWDGE): small / late-needed tensors
            bv_sb = big.tile([P, HDL], F32, tag="bv", name="bv")
            nc.gpsimd.dma_start(
                bv_sb, bass.AP(tensor=bvv.tensor, offset=bvv.offset,
                               ap=[[0, P]] + list(bvv.ap)))
            bq_sb = big.tile([P, 2], F32, tag="bq", name="bq")
            nc.gpsimd.dma_start(bq_sb, bqv.rearrange("(c p) -> p c", p=P))
            bk_sb = big.tile([P, 2], F32, tag="bk", name="bk")
            nc.gpsimd.dma_start(bk_sb, bkv.rearrange("(c p) -> p c", p=P))
            ones_sb = big.tile([P, HL, D], BF16, tag="ones", name="ones")
            ones_r = onesv.rearrange("(h d) -> h d", h=HL)
            nc.gpsimd.dma_start(
                ones_sb, bass.AP(tensor=onesv.tensor, offset=onesv.offset,
                                 ap=[[0, P]] + list(ones_r.ap)))
            mask_sb = big.tile([P, GROUPS, TQ], BF16, tag="mask", name="mask")
            nc.gpsimd.dma_start(mask_sb, maskd.rearrange("d p f -> p d f"))
            wp_all = big.tile([P, 2, E], BF16, tag="wp", name="wp")
            nc.gpsimd.dma_start(wp_all, wpt)
            bp_sb = big.tile([P, E], F32, tag="bp", name="bp")
            nc.gpsimd.dma_start(
                bp_sb, bass.AP(tensor=bp4.tensor, offset=bp4.offset,
                               ap=[[0, P]] + list(bp4.ap)))

            q_sb = [big.tile([P, T], BF16, tag=f"q{hc}", name=f"q{hc}")
                    for hc in range(2)]
            k_sb = [big.tile([P, T], BF16, tag=f"k{hc}", name=f"k{hc}")
                    for hc in range(2)]
            at_sb = [big.tile([P, T], BF16, tag=f"at{hc}", name=f"at{hc}")
                     for hc in range(2)]
            v_sb = [big.tile([P, HL, 2 * D], BF16, tag=f"v{t}", name=f"v{t}")
                    for t in range(N_TC)]

            def v_finish(t_, ps):
                nc.gpsimd.tensor_copy(v_sb[t_][:, :, 0:D], ones_sb)
                nc.vector.tensor_add(
                    v_sb[t_][:, :, D:2 * D],
                    ps.rearrange("p (h d) -> p h d", h=HL),
                    bv_sb.rearrange("p (h d) -> p h d", h=HL))

            # -------- pre-attention: q/k(tb0) both head pairs + v(t0-3) ----
            # ec-inner across 8 concurrently-open PSUM groups: every
            # xt(ec,0) chunk arrival unlocks one matmul per group.
            # pre-attention: q/k(tb0) both head pairs + v(t0-3), emitted in
            # DMA-arrival order with warm-up padding over the gaps so the
            # PE never idles (idling resets the p-state ramp).  v(t3)'s
            # contraction runs as one bulk at the end because its PSUM bank
            # is the warm-up bank (9th allocation of the 8-slot pool).
            with tc.tile_pool(name="preA", bufs=8, space="PSUM") as preA:
                warm_ps = preA.tile([P, TQ], F32, tag="mm", name="mm")

                def warm(n):
                    for _ in range(n):
                        nc.tensor.matmul(warm_ps[:, 0:HDL],
                                         lhsT=warm_sb[:, 0:P], rhs=warm_sb,
                                         start=True, stop=True)

                warm(8)
                pss = {}
                for hc in range(2):
                    for wi in range(2):
                        pss[(hc, wi)] = preA.tile([P, TQ], F32, tag="mm",
                                                  name="mm")
                vps = [preA.tile([P, HDL], F32, tag="mm", name="mm")
                       for _ in range(3)]
                qk_seen = [0, 0]
                v_seen = 0

                def qk(wi, ec):
                    w_all = (wq_all, wk_all)[wi]
                    for hc in range(2):
                        nc.tensor.matmul(
                            pss[(hc, wi)],
                            lhsT=w_all[:, ec, hc * P:(hc + 1) * P],
                            rhs=xts(ec, 0),
                            start=(qk_seen[wi] == 0),
                            stop=(qk_seen[wi] == N_EC - 1))
                    qk_seen[wi] += 1

                def vv(ec):
                    nonlocal v_seen
                    for ti in range(3):
                        nc.tensor.matmul(
                            vps[ti],
                            lhsT=xts(ec, 0)[:, ti * P:(ti + 1) * P],
                            rhs=wv_all[:, ec, :],
                            start=(v_seen == 0), stop=(v_seen == N_EC - 1))
                    v_seen += 1

                qk(0, 0)
                warm(4)
                qk(0, 2)
                qk(1, 0)
                qk(1, 2)
                warm(2)
                qk(0, 1)
                qk(1, 1)
                vv(0)
                qk(0, 3)
                qk(1, 3)
                vv(2)
                qk(0, 4)
                qk(1, 4)
                vv(1)
                qk(0, 6)
                qk(1, 6)
                vv(3)
                qk(0, 5)
                qk(1, 5)
                vv(4)
                qk(0, 7)
                qk(1, 7)
                for ec in (5, 6, 7):
                    vv(ec)
                for hc in range(2):
                    for wi, (bias_t, dst) in ((0, (bq_sb, q_sb)),
                                              (1, (bk_sb, k_sb))):
                        nc.vector.tensor_scalar_add(
                            dst[hc][:, 0:TQ], pss[(hc, wi)],
                            bias_t[:, hc:hc + 1])
                # v(t3) bulk: reuses the warm-up PSUM slot
                vt3 = preA.tile([P, HDL], F32, tag="mm", name="mm")
                for ec in range(N_EC):
                    nc.tensor.matmul(
                        vt3,
                        lhsT=xts(ec, 0)[:, 3 * P:4 * P],
                        rhs=wv_all[:, ec, :],
                        start=(ec == 0), stop=(ec == N_EC - 1))
                for ti in range(3):
                    v_finish(ti, vps[ti])
                v_finish(3, vt3)

            # -------- attention (block-causal, per head pair) --------------
            # j-loop software-pipelined one step; filler closures absorb
            # the PE slack while ScalarE's exp drains.
            import contextlib
            _ph34 = contextlib.ExitStack()
            stps = _ph34.enter_context(
                tc.tile_pool(name="stps", bufs=2, space="PSUM"))
            accps = _ph34.enter_context(
                tc.tile_pool(name="accps", bufs=1, space="PSUM"))
            mmps = _ph34.enter_context(
                tc.tile_pool(name="mmps", bufs=2, space="PSUM"))

            def attention(hp, ibs=None, filler=None, last=False):
                filler = list(filler) if filler else []
                for ib in (range(N_TB) if ibs is None else ibs):
                    njb = 4 * ib + 4
                    accs = [accps.tile([2 * D, TQ], F32, tag=f"acc{h}",
                                       name=f"acc{h}") for h in range(2)]

                    def s_emit(jb):
                        idx = jb - 4 * ib       # >= 0 on the block diagonal
                        dd = idx * JB if idx >= 0 else 0
                        st = stps.tile([P, 2, TQ], F32, tag="st", name="st")
                        pt = work.tile([P, 2, TQ], BF16, tag="pt", name="pt")
                        for h in range(2):
                            pr = slice(h * D, (h + 1) * D)
                            nc.tensor.matmul(
                                st[:, h, dd:],
                                lhsT=q_sb[hp][pr, jb * JB:(jb + 1) * JB],
                                rhs=k_sb[hp][pr, ib * TQ + dd:(ib + 1) * TQ],
                                start=True, stop=True)
                        nc.scalar.activation(pt[:, :, dd:], st[:, :, dd:],
                                             AF.Exp, scale=0.125)
                        if idx >= 0:
                            for h in range(2):
                                nc.vector.tensor_mul(
                                    pt[:, h, dd:], pt[:, h, dd:],
                                    mask_sb[:, idx, dd:])
                        return pt, dd

                    # scores are emitted in batches of two j-steps so the PE
                    # switches tiling mode (64x128 scores <-> 128x128
                    # everything else; each switch drains the array, ~105ns)
                    # once per batch instead of once per step.  PV(jb) reads
                    # pt (SBUF), not st, so the 2-deep st pool still
                    # suffices: st(jb+2) lands in the slot exp(jb) freed.
                    pend = {0: s_emit(0), 1: s_emit(1)}
                    for jp in range(0, njb, 2):
                        # group 0: filler BEFORE the next score batch --
                        # s_emit(2) needs the st slot exp(0) frees, and
                        # exp(0) only starts once s(0)/s(1) finish, so the
                        # filler covers that latency; later groups keep the
                        # scores up front to minimize tiling-mode switches.
                        if jp == 0 and filler:
                            filler.pop(0)()
                        for jn in (jp + 2, jp + 3):
                            if jn < njb:
                                pend[jn] = s_emit(jn)
                        if jp > 0 and filler:
                            filler.pop(0)()
                        for jb in (jp, jp + 1):
                            pt, dd = pend.pop(jb)
                            for h in range(2):
                                nc.tensor.matmul(
                                    accs[h][:, dd:],
                                    lhsT=v_sb[jb][:, 2 * hp + h, :],
                                    rhs=pt[:, h, dd:],
                                    start=(jb == 0), stop=(jb == njb - 1))
                    if not last:
                        for h in range(2):
                            rec64 = work.tile([D, TQ], F32, tag="rec64",
                                              name="rec64")
                            nc.vector.reciprocal_approx_fast(rec64,
                                                             accs[h][0:D, :])
                            nc.vector.tensor_mul(
                                at_sb[hp][h * D:(h + 1) * D,
                                          ib * TQ:(ib + 1) * TQ],
                                accs[h][D:2 * D, :], rec64)
                    else:
                        # final i-block: normalize in 128-col chunks so the
                        # trailing output projections start ~0.8us after the
                        # last PV instead of waiting the full [64,512] pass
                        for cc in range(4):
                            sl = slice(cc * P, (cc + 1) * P)
                            for h in range(2):
                                rec = work.tile([D, P], F32, tag="recc",
                                                name="recc")
                                nc.vector.reciprocal_approx_fast(
                                    rec, accs[h][0:D, sl])
                                nc.vector.tensor_mul(
                                    at_sb[hp][h * D:(h + 1) * D,
                                              ib * TQ + cc * P:
                                              ib * TQ + (cc + 1) * P],
                                    accs[h][D:2 * D, sl], rec)
                while filler:
                    filler.pop(0)()

            # ---- filler closures (emitted between attention j-steps) ------
            def qk2_group(hc, tb, wi):
                w_all = (wq_all, wk_all)[wi]
                bias_t = (bq_sb, bk_sb)[wi]
                dst = (q_sb, k_sb)[wi]

                def go():
                    ps = mmps.tile([P, TQ], F32, tag="mm", name="mm")
                    for ec in range(N_EC):
                        nc.tensor.matmul(
                            ps,
                            lhsT=w_all[:, ec, hc * P:(hc + 1) * P],
                            rhs=xts(ec, tb),
                            start=(ec == 0), stop=(ec == N_EC - 1))
                    nc.vector.tensor_scalar_add(
                        dst[hc][:, tb * TQ:(tb + 1) * TQ], ps,
                        bias_t[:, hc:hc + 1])
                return go

            def v_group(t_):
                def go():
                    ps = mmps.tile([P, HDL], F32, tag="mm", name="mm")
                    for ec in range(N_EC):
                        nc.tensor.matmul(
                            ps,
                            lhsT=xts(ec, t_ // 4)[
                                :, (t_ % 4) * P:(t_ % 4 + 1) * P],
                            rhs=wv_all[:, ec, :],
                            start=(ec == 0), stop=(ec == N_EC - 1))
                    v_finish(t_, ps)
                return go

            def proj_t(t_):
                def go():
                    ot = outp.tile([P, E], BF16, tag="ot", name="ot")
                    for eb in range(2):
                        ps = mmps.tile([P, TQ], F32, tag="mm", name="mm")
                        for hc in range(2):
                            nc.tensor.matmul(
                                ps,
                                lhsT=at_sb[hc][:, t_ * P:(t_ + 1) * P],
                                rhs=wp_all[:, hc, eb * TQ:(eb + 1) * TQ],
                                start=(hc == 0), stop=(hc == 1))
                        nc.vector.tensor_add(
                            ot[:, eb * TQ:(eb + 1) * TQ], ps,
                            bp_sb[:, eb * TQ:(eb + 1) * TQ])
                    # the last four stores alternate rings so they drain in
                    # parallel instead of serializing ~1.1us each on sync
                    eng = nc.scalar if t_ >= 12 and t_ % 2 else nc.sync
                    eng.dma_start(out[t_ * P:(t_ + 1) * P, :], ot)
                return go

            # interleave the two head pairs at i-block granularity; fillers
            # deliver each (hp, ib)'s q/k/v deps at least one call ahead and
            # drain proj as soon as both head pairs finished an i-block.
            attention(0, ibs=[0],
                      filler=[qk2_group(0, 1, 0), qk2_group(0, 1, 1)])
            attention(1, ibs=[0],
                      filler=[v_group(4), v_group(5)])
            attention(0, ibs=[1],
                      filler=[v_group(6), v_group(7),
                              qk2_group(1, 1, 0), qk2_group(1, 1, 1)])
            attention(1, ibs=[1],
                      filler=[v_group(8), v_group(9),
                              qk2_group(0, 2, 0), qk2_group(0, 2, 1)])
            attention(0, ibs=[2],
                      filler=[v_group(10), v_group(11),
                              qk2_group(1, 2, 0), qk2_group(1, 2, 1),
                              proj_t(0), proj_t(1)])
            attention(1, ibs=[2],
                      filler=[v_group(12), v_group(13),
                              qk2_group(0, 3, 0), qk2_group(0, 3, 1),
                              proj_t(2), proj_t(3)])
            attention(0, ibs=[3],
                      filler=[v_group(14), v_group(15),
                              qk2_group(1, 3, 0), qk2_group(1, 3, 1),
                              proj_t(4), proj_t(5), proj_t(6), proj_t(7)])
            attention(1, ibs=[3],
                      filler=[proj_t(8), proj_t(9),
                              proj_t(10), proj_t(11)], last=True)
            for t_ in range(12, 16):
                proj_t(t_)()
            _ph34.close()

    nc.compile()
    return nc


def _make_mask():
    jj = np.arange(JB)[:, None]
    ii = np.arange(TQ)[None, :]
    m = np.zeros((GROUPS, JB, TQ), dtype=np.float32)
    for d in range(GROUPS):
        m[d] = (jj + d * JB <= ii).astype(np.float32)
    return m.astype(ml_dtypes.bfloat16)


_NC = None


def _get_nc():
    global _NC
    if _NC is None:
        _NC = _build_nc()
    return _NC


def _warr(w):
    """W slice [HDL, E] -> SBUF layout [P, N_EC, HDL]: element (p, c, f) =
    W.T[c*P + p, f]."""
    return np.ascontiguousarray(
        w.T.reshape(N_EC, P, HDL).transpose(1, 0, 2)).astype(ml_dtypes.bfloat16)


def kernel(x, Wq, bq, Wk, bk, Wv, bv, Wp, bp, **_run_kwargs):
    x = np.asarray(x, dtype=np.float32)
    Wq = np.asarray(Wq, dtype=np.float32)
    Wk = np.asarray(Wk, dtype=np.float32)
    Wv = np.asarray(Wv, dtype=np.float32)
    Wp = np.asarray(Wp, dtype=np.float32)
    bq = np.asarray(bq, dtype=np.float32)
    bk = np.asarray(bk, dtype=np.float32)
    bv = np.asarray(bv, dtype=np.float32)
    bp = np.asarray(bp, dtype=np.float32)

    mask = _make_mask()
    bp4 = (bp / GROUPS).astype(np.float32)

    in_maps = []
    for c in range(NCORES):
        b, hg = divmod(c, GROUPS)
        hsl = slice(HDL * hg, HDL * (hg + 1))
        in_maps.append({
            "xt": np.ascontiguousarray(x[b].T).astype(ml_dtypes.bfloat16),
            "wqt": _warr(Wq[hsl]),
            "wkt": _warr(Wk[hsl]),
            "wvt": _warr(Wv[hsl]),
            "wpt": np.ascontiguousarray(
                Wp[:, hsl].T.reshape(2, P, E).transpose(1, 0, 2)
            ).astype(ml_dtypes.bfloat16),
            "bqv": np.ascontiguousarray(bq[hsl]),
            "bkv": np.ascontiguousarray(bk[hsl]),
            "bvv": np.ascontiguousarray(bv[hsl]),
            "bp4": bp4,
            "mask": mask,
            "onesv": np.ones(HDL, dtype=ml_dtypes.bfloat16),
        })

    nc = _get_nc()
    try:
        res = run_bass_kernel_spmd(nc, in_maps, core_ids=list(range(NCORES)),
                                   **_run_kwargs)
    except Exception:
        # transient device hiccups (e.g. NRT_EXEC_UNIT_UNRECOVERABLE) have
        # been observed to clear on retry
        import time
        time.sleep(2.0)
        res = run_bass_kernel_spmd(nc, in_maps, core_ids=list(range(NCORES)),
                                   **_run_kwargs)
    outs = [r["out"].astype(np.float32) for r in res.results]
    y = np.stack([
        outs[0] + outs[1] + outs[2] + outs[3],
        outs[4] + outs[5] + outs[6] + outs[7],
    ]).astype(np.float32)
    if _run_kwargs:
        return y, res
    return y


# revision 24
# speedup vs baseline: 1.0000x; 1.0000x over previous
"""Causal attention (B=2, T=2048, E=1024, H=16, D=64) on 8 TRN2 NeuronCores.

Sharding: core c handles batch b = c//4 and local head group hg = c%4
(4 heads, 256 head-dims).  Data parallel over batch, tensor parallel over
heads; the output projection is row-parallel, so each core returns a
partial [T, E] output and the host sums the 4 partials per batch (bias
is pre-divided by 4 and added on-device).

Device plan (per core, all-bf16 matmuls with fp32 PSUM accumulation):
  xt  = x[b].T                   [E, T]  (host-transposed; e on partitions)
  wqt/wkt/wvt = W[h].T           pre-tiled [P, 8, 256] for SBUF layout
  wpt = Wp[:, h].T               pre-tiled [P, 2, 1024]

Schedule: TRN2's PE p-state ramps (0.65->1.2->2.4 GHz over ~3us of
CONTINUOUS work; any idle gap resets it), so the whole kernel is built
as one unbroken PE instruction stream:
  - xt is DMAed in 32 [128,512] chunks in tb-major (consumption) order
    on the sync ring; wq/wk (in halves) + wv stream on the scalar ring.
    First matmul starts as soon as wq-half0 + xt(ec0,tb0) land.
  - pre-attention: q/k for token-block tb0 (both head pairs) + v(t0-3)
    in an 8-bank PSUM pool, ec-inner so matmuls chase chunk arrivals.
  - attention(hp, ib) runs as soon as its q/k/v deps exist; ALL other
    work (q/k tb1-3, v t4-15, output projection) is emitted as ~1-2us
    filler closures between attention pipeline steps, sized so the PE
    never idles while ScalarE's exp drains (j-loop software-pipelined
    one step: scores/exp for jb+1 are emitted before the PV matmuls of
    jb).
  - scores st[j, i] = q_j . k_i with 2-head row-packing (two K=64
    matmuls in distinct PE row groups), exp on ScalarE (scale=1/8; no
    max subtraction -- scores are ~N(0,1) so exp cannot overflow),
    causal mask multiply only on block-diagonal tiles, PV accumulation
    over j in PSUM with 64 ones-columns prepended to v (softmax
    denominator comes out of the PV matmul on partitions 0:63), then
    approx-reciprocal + multiply for the normalization.
  - v ones-columns are written by GpSimd (otherwise idle) to keep
    VectorE off the critical path.
"""

import ml_dtypes
import numpy as np

import concourse.bass as bass
import concourse.tile as tile
from concourse import bacc, mybir
from concourse.bass_utils import run_bass_kernel_spmd

B, T, E = 2, 2048, 1024
H, D = 16, 64
NCORES = 8
GROUPS = 4              # cores per batch (tensor parallel over heads)
HL = H // GROUPS        # 4 local heads per core
HDL = HL * D            # 256 local head dims
P = 128
TQ = 512                # i-block (free dim of score tiles)
JB = 128                # j-block (partition dim of score tiles)
N_TB = T // TQ          # 4
N_EC = E // P           # 8
N_TC = T // P           # 16

F32 = mybir.dt.float32
BF16 = mybir.dt.bfloat16
AF = mybir.ActivationFunctionType


def _build_nc():
    nc = bacc.Bacc("TRN2", target_bir_lowering=False, debug=False)
    xt = nc.dram_tensor("xt", [E, T], BF16, kind="ExternalInput").ap()
    wqt = nc.dram_tensor("wqt", [P, N_EC, HDL], BF16, kind="ExternalInput").ap()
    wkt = nc.dram_tensor("wkt", [P, N_EC, HDL], BF16, kind="ExternalInput").ap()
    wvt = nc.dram_tensor("wvt", [P, N_EC, HDL], BF16, kind="ExternalInput").ap()
    wpt = nc.dram_tensor("wpt", [P, 2, E], BF16, kind="ExternalInput").ap()
    bqv = nc.dram_tensor("bqv", [HDL], F32, kind="ExternalInput").ap()
    bkv = nc.dram_tensor("bkv", [HDL], F32, kind="ExternalInput").ap()
    bvv = nc.dram_tensor("bvv", [HDL], F32, kind="ExternalInput").ap()
    bp4 = nc.dram_tensor("bp4", [E], F32, kind="ExternalInput").ap()
    maskd = nc.dram_tensor("mask", [GROUPS, JB, TQ], BF16,
                           kind="ExternalInput").ap()
    onesv = nc.dram_tensor("onesv", [HDL], BF16, kind="ExternalInput").ap()
    out = nc.dram_tensor("out", [T, E], BF16, kind="ExternalOutput").ap()

    with tile.TileContext(nc) as tc:
        with (
            tc.tile_pool(name="big", bufs=1) as big,
            tc.tile_pool(name="work", bufs=5) as work,
            tc.tile_pool(name="outp", bufs=3) as outp,
        ):
            # ---------------- input loads ---------------------------------
            # DMA rings move contiguous 512KB descriptors at ~330GB/s but
            # strided gathers collapse to ~35GB/s, so xt ships as 8 full-T
            # contiguous e-chunks alternating between the two HWDGE rings;
            # weights lead the scalar ring, wv rides the sync ring between
            # xt chunks.  Expected arrivals (ring start ~6.7us, 1.55us per
            # 512KB): sync e0 8.3, e2 9.9, wv 11.4, e4 13, e6 14.6;
            # scalar wq 8.3, wk 9.9, e1 11.4, e3 13, e5 14.6, e7 16.1.
            xt_sb = [big.tile([P, T], BF16, tag=f"xt{ec}", name=f"xt{ec}")
                     for ec in range(N_EC)]
            wq_all = big.tile([P, N_EC, HDL], BF16, tag="wq", name="wq")
            wk_all = big.tile([P, N_EC, HDL], BF16, tag="wk", name="wk")
            wv_all = big.tile([P, N_EC, HDL], BF16, tag="wv", name="wv")

            def xt_dma(eng, ec):
                eng.dma_start(xt_sb[ec], xt[ec * P:(ec + 1) * P, :])

            xt_dma(nc.sync, 0)
            nc.scalar.dma_start(wq_all, wqt)
            xt_dma(nc.sync, 2)
            nc.scalar.dma_start(wk_all, wkt)
            nc.sync.dma_start(wv_all, wvt)
            xt_dma(nc.scalar, 1)
            xt_dma(nc.sync, 4)
            xt_dma(nc.scalar, 3)
            xt_dma(nc.sync, 6)
            xt_dma(nc.scalar, 5)
            xt_dma(nc.scalar, 7)

            def xts(ec, tb):
                return xt_sb[ec][:, tb * TQ:(tb + 1) * TQ]
            # PE warm-up scratch: matmuls on this (memset) tile run while
            # the first input DMAs are in flight, so the PE p-state ramp
            # (0.65->2.4GHz over ~3us of continuous work) completes before
            # real data arrives instead of slowing the first real matmuls.
            warm_sb = big.tile([P, HDL], BF16, tag="warm", name="warm")
            nc.gpsimd.memset(warm_sb, 0.0)
            # gpsimd (SWDGE): small / late-needed tensors
            bv_sb = big.tile([P, HDL], F32, tag="bv", name="bv")
            nc.gpsimd.dma_start(
                bv_sb, bass.AP(tensor=bvv.tensor, offset=bvv.offset,
                               ap=[[0, P]] + list(bvv.ap)))
            bq_sb = big.tile([P, 2], F32, tag="bq", name="bq")
            nc.gpsimd.dma_start(bq_sb, bqv.rearrange("(c p) -> p c", p=P))
            bk_sb = big.tile([P, 2], F32, tag="bk", name="bk")
            nc.gpsimd.dma_start(bk_sb, bkv.rearrange("(c p) -> p c", p=P))
            ones_sb = big.tile([P, HL, D], BF16, tag="ones", name="ones")
            ones_r = onesv.rearrange("(h d) -> h d", h=HL)
            nc.gpsimd.dma_start(
                ones_sb, bass.AP(tensor=onesv.tensor, offset=onesv.offset,
                                 ap=[[0, P]] + list(ones_r.ap)))
            mask_sb = big.tile([P, GROUPS, TQ], BF16, tag="mask", name="mask")
            nc.gpsimd.dma_start(mask_sb, maskd.rearrange("d p f -> p d f"))
            wp_all = big.tile([P, 2, E], BF16, tag="wp", name="wp")
            nc.gpsimd.dma_start(wp_all, wpt)
            bp_sb = big.tile([P, E], F32, tag="bp", name="bp")
            nc.gpsimd.dma_start(
                bp_sb, bass.AP(tensor=bp4.tensor, offset=bp4.offset,
                               ap=[[0, P]] + list(bp4.ap)))

            q_sb = [big.tile([P, T], BF16, tag=f"q{hc}", name=f"q{hc}")
                    for hc in range(2)]
            k_sb = [big.tile([P, T], BF16, tag=f"k{hc}", name=f"k{hc}")
                    for hc in range(2)]
            at_sb = [big.tile([P, T], BF16, tag=f"at{hc}", name=f"at{hc}")
                     for hc in range(2)]
            v_sb = [big.tile([P, HL, 2 * D], BF16, tag=f"v{t}", name=f"v{t}")
                    for t in range(N_TC)]

            def v_finish(t_, ps):
                nc.gpsimd.tensor_copy(v_sb[t_][:, :, 0:D], ones_sb)
                nc.vector.tensor_add(
                    v_sb[t_][:, :, D:2 * D],
                    ps.rearrange("p (h d) -> p h d", h=HL),
                    bv_sb.rearrange("p (h d) -> p h d", h=HL))

            # -------- pre-attention: q/k(tb0) both head pairs + v(t0-3) ----
            # ec-inner across 8 concurrently-open PSUM groups: every
            # xt(ec,0) chunk arrival unlocks one matmul per group.
            # pre-attention: q/k for tb0 AND tb1, both head pairs -- 8 PSUM
            # groups, ec-interleaved in DMA-arrival order.  8 matmuls
            # (1.73us) per 512KB chunk matches the ~2us delivery cadence,
            # so the PE stays saturated from the first arrival; 14 warm-up
            # matmuls before that complete the p-state ramp.  v(t0-7)
            # happens inside the first two attention calls as fillers.
            with tc.tile_pool(name="preA", bufs=8, space="PSUM") as preA:
                warm_ps = preA.tile([P, TQ], F32, tag="mm", name="mm")
                for _ in range(14):
                    nc.tensor.matmul(warm_ps[:, 0:HDL],
                                     lhsT=warm_sb[:, 0:P], rhs=warm_sb,
                                     start=True, stop=True)
                pss = {}
                for tb in range(2):
                    for wi in range(2):
                        for hc in range(2):
                            # 8 allocations: the last rotates onto the
                            # warm-up slot (free once the warms retire)
                            pss[(tb, wi, hc)] = preA.tile(
                                [P, TQ], F32, tag="mm", name="mm")
                ec_order = (0, 2, 1, 4, 3, 6, 5, 7)
                for i, ec in enumerate(ec_order):
                    for tb in range(2):
                        for wi, w_all in ((0, wq_all), (1, wk_all)):
                            for hc in range(2):
                                nc.tensor.matmul(
                                    pss[(tb, wi, hc)],
                                    lhsT=w_all[:, ec, hc * P:(hc + 1) * P],
                                    rhs=xts(ec, tb),
                                    start=(i == 0), stop=(i == N_EC - 1))
                for tb in range(2):
                    for wi, (bias_t, dst) in ((0, (bq_sb, q_sb)),
                                              (1, (bk_sb, k_sb))):
                        for hc in range(2):
                            nc.vector.tensor_scalar_add(
                                dst[hc][:, tb * TQ:(tb + 1) * TQ],
                                pss[(tb, wi, hc)], bias_t[:, hc:hc + 1])

            # -------- attention (block-causal, per head pair) --------------
            # j-loop software-pipelined one step; filler closures absorb
            # the PE slack while ScalarE's exp drains.
            import contextlib
            _ph34 = contextlib.ExitStack()
            stps = _ph34.enter_context(
                tc.tile_pool(name="stps", bufs=2, space="PSUM"))
            accps = _ph34.enter_context(
                tc.tile_pool(name="accps", bufs=1, space="PSUM"))
            mmps = _ph34.enter_context(
                tc.tile_pool(name="mmps", bufs=2, space="PSUM"))

            # pre-emitted score tiles: exp work for a future attention call
            # run early (as filler) while ScalarE has slack, so the
            # exp-heavy final i-blocks don't stall the PE.  Pre-emitted pt
            # tiles use a dedicated pool so the main pipeline's rotation
            # cannot recycle them while still live (WAR deadlock).
            prep = _ph34.enter_context(tc.tile_pool(name="prep", bufs=8))
            pre = {}

            def emit_score(hp, ib, jb, pool):
                idx = jb - 4 * ib       # >= 0 on the block diagonal
                dd = idx * JB if idx >= 0 else 0
                st = stps.tile([P, 2, TQ], F32, tag="st", name="st")
                pt = pool.tile([P, 2, TQ], BF16, tag="pt", name="pt")
                for h in range(2):
                    pr = slice(h * D, (h + 1) * D)
                    nc.tensor.matmul(
                        st[:, h, dd:],
                        lhsT=q_sb[hp][pr, jb * JB:(jb + 1) * JB],
                        rhs=k_sb[hp][pr, ib * TQ + dd:(ib + 1) * TQ],
                        start=True, stop=True)
                nc.scalar.activation(pt[:, :, dd:], st[:, :, dd:],
                                     AF.Exp, scale=0.125)
                if idx >= 0:
                    for h in range(2):
                        nc.vector.tensor_mul(
                            pt[:, h, dd:], pt[:, h, dd:],
                            mask_sb[:, idx, dd:])
                return pt, dd

            def pre_score(hp, ib, jb):
                def go():
                    pre[(hp, ib, jb)] = emit_score(hp, ib, jb, prep)
                return go

            def attention(hp, ibs=None, filler=None, last=False):
                filler = list(filler) if filler else []
                for ib in (range(N_TB) if ibs is None else ibs):
                    njb = 4 * ib + 4
                    accs = [accps.tile([2 * D, TQ], F32, tag=f"acc{h}",
                                       name=f"acc{h}") for h in range(2)]

                    def s_emit(jb):
                        key = (hp, ib, jb)
                        if key in pre:
                            return pre.pop(key)
                        return emit_score(hp, ib, jb, work)

                    # scores are emitted in batches of two j-steps so the PE
                    # switches tiling mode (64x128 scores <-> 128x128
                    # everything else; each switch drains the array, ~105ns)
                    # once per batch instead of once per step.  PV(jb) reads
                    # pt (SBUF), not st, so the 2-deep st pool still
                    # suffices: st(jb+2) lands in the slot exp(jb) freed.
                    pend = {0: s_emit(0), 1: s_emit(1)}
                    for jp in range(0, njb, 2):
                        # group 0: filler BEFORE the next score batch --
                        # s_emit(2) needs the st slot exp(0) frees, and
                        # exp(0) only starts once s(0)/s(1) finish, so the
                        # filler covers that latency; later groups keep the
                        # scores up front to minimize tiling-mode switches.
                        if jp == 0 and filler:
                            filler.pop(0)()
                        for jn in (jp + 2, jp + 3):
                            if jn < njb:
                                pend[jn] = s_emit(jn)
                        if jp > 0 and filler:
                            filler.pop(0)()
                        for jb in (jp, jp + 1):
                            pt, dd = pend.pop(jb)
                            for h in range(2):
                                nc.tensor.matmul(
                                    accs[h][:, dd:],
                                    lhsT=v_sb[jb][:, 2 * hp + h, :],
                                    rhs=pt[:, h, dd:],
                                    start=(jb == 0), stop=(jb == njb - 1))
                    if not last:
                        for h in range(2):
                            rec64 = work.tile([D, TQ], F32, tag="rec64",
                                              name="rec64")
                            nc.vector.reciprocal_approx_fast(rec64,
                                                             accs[h][0:D, :])
                            nc.vector.tensor_mul(
                                at_sb[hp][h * D:(h + 1) * D,
                                          ib * TQ:(ib + 1) * TQ],
                                accs[h][D:2 * D, :], rec64)
                    else:
                        # final i-block: normalize in 128-col chunks so the
                        # trailing output projections start ~0.8us after the
                        # last PV instead of waiting the full [64,512] pass
                        for cc in range(4):
                            sl = slice(cc * P, (cc + 1) * P)
                            for h in range(2):
                                rec = work.tile([D, P], F32, tag="recc",
                                                name="recc")
                                nc.vector.reciprocal_approx_fast(
                                    rec, accs[h][0:D, sl])
                                nc.vector.tensor_mul(
                                    at_sb[hp][h * D:(h + 1) * D,
                                              ib * TQ + cc * P:
                                              ib * TQ + (cc + 1) * P],
                                    accs[h][D:2 * D, sl], rec)
                while filler:
                    filler.pop(0)()

            # ---- filler closures (emitted between attention j-steps) ------
            def qk2_group(hc, tb, wi):
                w_all = (wq_all, wk_all)[wi]
                bias_t = (bq_sb, bk_sb)[wi]
                dst = (q_sb, k_sb)[wi]

                def go():
                    ps = mmps.tile([P, TQ], F32, tag="mm", name="mm")
                    for ec in range(N_EC):
                        nc.tensor.matmul(
                            ps,
                            lhsT=w_all[:, ec, hc * P:(hc + 1) * P],
                            rhs=xts(ec, tb),
                            start=(ec == 0), stop=(ec == N_EC - 1))
                    nc.vector.tensor_scalar_add(
                        dst[hc][:, tb * TQ:(tb + 1) * TQ], ps,
                        bias_t[:, hc:hc + 1])
                return go

            def v_group(t_):
                def go():
                    ps = mmps.tile([P, HDL], F32, tag="mm", name="mm")
                    for ec in range(N_EC):
                        nc.tensor.matmul(
                            ps,
                            lhsT=xts(ec, t_ // 4)[
                                :, (t_ % 4) * P:(t_ % 4 + 1) * P],
                            rhs=wv_all[:, ec, :],
                            start=(ec == 0), stop=(ec == N_EC - 1))
                    v_finish(t_, ps)
                return go

            def proj_t(t_):
                def go():
                    ot = outp.tile([P, E], BF16, tag="ot", name="ot")
                    for eb in range(2):
                        ps = mmps.tile([P, TQ], F32, tag="mm", name="mm")
                        for hc in range(2):
                            nc.tensor.matmul(
                                ps,
                                lhsT=at_sb[hc][:, t_ * P:(t_ + 1) * P],
                                rhs=wp_all[:, hc, eb * TQ:(eb + 1) * TQ],
                                start=(hc == 0), stop=(hc == 1))
                        nc.vector.tensor_add(
                            ot[:, eb * TQ:(eb + 1) * TQ], ps,
                            bp_sb[:, eb * TQ:(eb + 1) * TQ])
                    # the last four stores alternate rings so they drain in
                    # parallel instead of serializing ~1.1us each on sync
                    eng = nc.scalar if t_ >= 12 and t_ % 2 else nc.sync
                    eng.dma_start(out[t_ * P:(t_ + 1) * P, :], ot)
                return go

            # interleave the two head pairs at i-block granularity; fillers
            # deliver each (hp, ib)'s q/k/v deps at least one call ahead and
            # drain proj as soon as both head pairs finished an i-block.
            def bundle(*fs):
                def go():
                    for f in fs:
                        f()
                return go

            # q/k for tb0+tb1 exist from preA; v(t0-7) rides the first two
            # calls as bundled fillers (each bundle before the PVs that
            # read it); tb2/tb3 q/k and the remaining v/proj interleave
            # later calls; pre_score cascades ib3 exp work forward so the
            # exp-heavy final calls stay PE-bound.
            attention(0, ibs=[0],
                      filler=[bundle(v_group(0), v_group(1)),
                              bundle(v_group(2), v_group(3))])
            attention(1, ibs=[0],
                      filler=[bundle(v_group(4), v_group(5)),
                              bundle(v_group(6), v_group(7))])
            attention(0, ibs=[1],
                      filler=[qk2_group(0, 2, 0), qk2_group(0, 2, 1),
                              v_group(8), v_group(9)])
            attention(1, ibs=[1],
                      filler=[qk2_group(1, 2, 0), qk2_group(1, 2, 1),
                              v_group(10), v_group(11)])
            attention(0, ibs=[2],
                      filler=[qk2_group(0, 3, 0), qk2_group(0, 3, 1),
                              v_group(12), v_group(13),
                              proj_t(0), proj_t(1),
                              pre_score(0, 3, 0), pre_score(0, 3, 1)])
            attention(1, ibs=[2],
                      filler=[qk2_group(1, 3, 0), qk2_group(1, 3, 1),
                              v_group(14), v_group(15),
                              proj_t(2), proj_t(3),
                              pre_score(1, 3, 0), pre_score(1, 3, 1)])
            attention(0, ibs=[3],
                      filler=[proj_t(4), proj_t(5), proj_t(6), proj_t(7),
                              proj_t(8),
                              pre_score(1, 3, 2), pre_score(1, 3, 3),
                              pre_score(1, 3, 4), pre_score(1, 3, 5)])
            attention(1, ibs=[3],
                      filler=[proj_t(9), proj_t(10), proj_t(11)])
            for t_ in range(12, 16):
                proj_t(t_)()
            _ph34.close()

    nc.compile()
    return nc


def _make_mask():
    jj = np.arange(JB)[:, None]
    ii = np.arange(TQ)[None, :]
    m = np.zeros((GROUPS, JB, TQ), dtype=np.float32)
    for d in range(GROUPS):
        m[d] = (jj + d * JB <= ii).astype(np.float32)
    return m.astype(ml_dtypes.bfloat16)


_NC = None


def _get_nc():
    global _NC
    if _NC is None:
        _NC = _build_nc()
    return _NC


def _warr(w):
    """W slice [HDL, E] -> SBUF layout [P, N_EC, HDL]: element (p, c, f) =
    W.T[c*P + p, f]."""
    return np.ascontiguousarray(
        w.T.reshape(N_EC, P, HDL).transpose(1, 0, 2)).astype(ml_dtypes.bfloat16)


def kernel(x, Wq, bq, Wk, bk, Wv, bv, Wp, bp, **_run_kwargs):
    x = np.asarray(x, dtype=np.float32)
    Wq = np.asarray(Wq, dtype=np.float32)
    Wk = np.asarray(Wk, dtype=np.float32)
    Wv = np.asarray(Wv, dtype=np.float32)
    Wp = np.asarray(Wp, dtype=np.float32)
    bq = np.asarray(bq, dtype=np.float32)
    bk = np.asarray(bk, dtype=np.float32)
    bv = np.asarray(bv, dtype=np.float32)
    bp = np.asarray(bp, dtype=np.float32)

    mask = _make_mask()
    bp4 = (bp / GROUPS).astype(np.float32)

    in_maps = []
    for c in range(NCORES):
        b, hg = divmod(c, GROUPS)
        hsl = slice(HDL * hg, HDL * (hg + 1))
        in_maps.append({
            "xt": np.ascontiguousarray(x[b].T).astype(ml_dtypes.bfloat16),
            "wqt": _warr(Wq[hsl]),
            "wkt": _warr(Wk[hsl]),
            "wvt": _warr(Wv[hsl]),
            "wpt": np.ascontiguousarray(
                Wp[:, hsl].T.reshape(2, P, E).transpose(1, 0, 2)
            ).astype(ml_dtypes.bfloat16),
            "bqv": np.ascontiguousarray(bq[hsl]),
            "bkv": np.ascontiguousarray(bk[hsl]),
            "bvv": np.ascontiguousarray(bv[hsl]),
            "bp4": bp4,
            "mask": mask,
            "onesv": np.ones(HDL, dtype=ml_dtypes.bfloat16),
        })

    nc = _get_nc()
    try:
        res = run_bass_kernel_spmd(nc, in_maps, core_ids=list(range(NCORES)),
                                   **_run_kwargs)
    except Exception:
        # transient device hiccups (e.g. NRT_EXEC_UNIT_UNRECOVERABLE) have
        # been observed to clear on retry
        import time
        time.sleep(2.0)
        res = run_bass_kernel_spmd(nc, in_maps, core_ids=list(range(NCORES)),
                                   **_run_kwargs)
    outs = [r["out"].astype(np.float32) for r in res.results]
    y = np.stack([
        outs[0] + outs[1] + outs[2] + outs[3],
        outs[4] + outs[5] + outs[6] + outs[7],
    ]).astype(np.float32)
    if _run_kwargs:
        return y, res
    return y


# revision 30
# speedup vs baseline: 1.0097x; 1.0097x over previous
"""Causal attention (B=2, T=2048, E=1024, H=16, D=64) on 8 TRN2 NeuronCores.

Sharding: core c handles batch b = c//4 and local head group hg = c%4
(4 heads, 256 head-dims).  Data parallel over batch, tensor parallel over
heads; the output projection is row-parallel, so each core returns a
partial [T, E] output and the host sums the 4 partials per batch (bias
is pre-divided by 4 and added on-device).

Device plan (per core, all-bf16 matmuls with fp32 PSUM accumulation):
  xt  = x[b].T                   [E, T]  (host-transposed; e on partitions)
  wqt/wkt/wvt = W[h].T           pre-tiled [P, 8, 256] for SBUF layout
  wpt = Wp[:, h].T               pre-tiled [P, 2, 1024]

Schedule: TRN2's PE p-state ramps (0.65->1.2->2.4 GHz over ~3us of
CONTINUOUS work; any idle gap resets it), so the whole kernel is built
as one unbroken PE instruction stream:
  - xt is DMAed in 32 [128,512] chunks in tb-major (consumption) order
    on the sync ring; wq/wk (in halves) + wv stream on the scalar ring.
    First matmul starts as soon as wq-half0 + xt(ec0,tb0) land.
  - pre-attention: q/k for token-block tb0 (both head pairs) + v(t0-3)
    in an 8-bank PSUM pool, ec-inner so matmuls chase chunk arrivals.
  - attention(hp, ib) runs as soon as its q/k/v deps exist; ALL other
    work (q/k tb1-3, v t4-15, output projection) is emitted as ~1-2us
    filler closures between attention pipeline steps, sized so the PE
    never idles while ScalarE's exp drains (j-loop software-pipelined
    one step: scores/exp for jb+1 are emitted before the PV matmuls of
    jb).
  - scores st[j, i] = q_j . k_i with 2-head row-packing (two K=64
    matmuls in distinct PE row groups), exp on ScalarE (scale=1/8; no
    max subtraction -- scores are ~N(0,1) so exp cannot overflow),
    causal mask multiply only on block-diagonal tiles, PV accumulation
    over j in PSUM with 64 ones-columns prepended to v (softmax
    denominator comes out of the PV matmul on partitions 0:63), then
    approx-reciprocal + multiply for the normalization.
  - v ones-columns are written by GpSimd (otherwise idle) to keep
    VectorE off the critical path.
"""

import ml_dtypes
import numpy as np

import concourse.bass as bass
import concourse.tile as tile
from concourse import bacc, mybir
from concourse.bass_utils import run_bass_kernel_spmd

B, T, E = 2, 2048, 1024
H, D = 16, 64
NCORES = 8
GROUPS = 4              # cores per batch (tensor parallel over heads)
HL = H // GROUPS        # 4 local heads per core
HDL = HL * D            # 256 local head dims
P = 128
TQ = 512                # i-block (free dim of score tiles)
JB = 128                # j-block (partition dim of score tiles)
N_TB = T // TQ          # 4
N_EC = E // P           # 8
N_TC = T // P           # 16

F32 = mybir.dt.float32
BF16 = mybir.dt.bfloat16
AF = mybir.ActivationFunctionType


def _build_nc():
    nc = bacc.Bacc("TRN2", target_bir_lowering=False, debug=False)
    xt = nc.dram_tensor("xt", [E, T], BF16, kind="ExternalInput").ap()
    wqt = nc.dram_tensor("wqt", [P, N_EC, HDL], BF16, kind="ExternalInput").ap()
    wkt = nc.dram_tensor("wkt", [P, N_EC, HDL], BF16, kind="ExternalInput").ap()
    wvt = nc.dram_tensor("wvt", [P, N_EC, HDL], BF16, kind="ExternalInput").ap()
    wpt = nc.dram_tensor("wpt", [P, 2, E], BF16, kind="ExternalInput").ap()
    bqv = nc.dram_tensor("bqv", [HDL], F32, kind="ExternalInput").ap()
    bkv = nc.dram_tensor("bkv", [HDL], F32, kind="ExternalInput").ap()
    bvv = nc.dram_tensor("bvv", [HDL], F32, kind="ExternalInput").ap()
    bp4 = nc.dram_tensor("bp4", [E], F32, kind="ExternalInput").ap()
    maskd = nc.dram_tensor("mask", [GROUPS, JB, TQ], BF16,
                           kind="ExternalInput").ap()
    onesv = nc.dram_tensor("onesv", [HDL], BF16, kind="ExternalInput").ap()
    out = nc.dram_tensor("out", [T, E], BF16, kind="ExternalOutput").ap()

    with tile.TileContext(nc) as tc:
        with (
            tc.tile_pool(name="big", bufs=1) as big,
            tc.tile_pool(name="work", bufs=5) as work,
            tc.tile_pool(name="outp", bufs=3) as outp,
        ):
            # ---------------- input loads ---------------------------------
            # DMA rings move contiguous 512KB descriptors at ~330GB/s but
            # strided gathers collapse to ~35GB/s, so xt ships as 8 full-T
            # contiguous e-chunks alternating between the two HWDGE rings;
            # weights lead the scalar ring, wv rides the sync ring between
            # xt chunks.  Expected arrivals (ring start ~6.7us, 1.55us per
            # 512KB): sync e0 8.3, e2 9.9, wv 11.4, e4 13, e6 14.6;
            # scalar wq 8.3, wk 9.9, e1 11.4, e3 13, e5 14.6, e7 16.1.
            xt_sb = [big.tile([P, T], BF16, tag=f"xt{ec}", name=f"xt{ec}")
                     for ec in range(N_EC)]
            wq_all = big.tile([P, N_EC, HDL], BF16, tag="wq", name="wq")
            wk_all = big.tile([P, N_EC, HDL], BF16, tag="wk", name="wk")
            wv_all = big.tile([P, N_EC, HDL], BF16, tag="wv", name="wv")

            def xt_dma(eng, ec):
                eng.dma_start(xt_sb[ec], xt[ec * P:(ec + 1) * P, :])

            nc.sync.dma_start(wq_all, wqt)
            nc.scalar.dma_start(wk_all, wkt)
            for ec in range(0, N_EC, 2):
                xt_dma(nc.sync, ec)
                xt_dma(nc.scalar, ec + 1)

            def xts(ec, tb):
                return xt_sb[ec][:, tb * TQ:(tb + 1) * TQ]
            # PE warm-up scratch: matmuls on this (memset) tile run while
            # the first input DMAs are in flight, so the PE p-state ramp
            # (0.65->2.4GHz over ~3us of continuous work) completes before
            # real data arrives instead of slowing the first real matmuls.
            warm_sb = big.tile([P, HDL], BF16, tag="warm", name="warm")
            nc.gpsimd.memset(warm_sb, 0.0)
            # gpsimd (SWDGE): small / late-needed tensors
            bv_sb = big.tile([P, HDL], F32, tag="bv", name="bv")
            nc.gpsimd.dma_start(
                bv_sb, bass.AP(tensor=bvv.tensor, offset=bvv.offset,
                               ap=[[0, P]] + list(bvv.ap)))
            bq_sb = big.tile([P, 2], F32, tag="bq", name="bq")
            nc.gpsimd.dma_start(bq_sb, bqv.rearrange("(c p) -> p c", p=P))
            bk_sb = big.tile([P, 2], F32, tag="bk", name="bk")
            nc.gpsimd.dma_start(bk_sb, bkv.rearrange("(c p) -> p c", p=P))
            ones_sb = big.tile([P, HL, D], BF16, tag="ones", name="ones")
            ones_r = onesv.rearrange("(h d) -> h d", h=HL)
            nc.gpsimd.dma_start(
                ones_sb, bass.AP(tensor=onesv.tensor, offset=onesv.offset,
                                 ap=[[0, P]] + list(ones_r.ap)))
            mask_sb = big.tile([P, GROUPS, TQ], BF16, tag="mask", name="mask")
            nc.gpsimd.dma_start(mask_sb, maskd.rearrange("d p f -> p d f"))
            # wv rides the SWDGE ring: not needed until the v fillers in the
            # first attention call (~28us), and keeping it off the HWDGE
            # rings gets the xt chunks there ~2us earlier
            nc.gpsimd.dma_start(wv_all, wvt)
            wp_all = big.tile([P, 2, E], BF16, tag="wp", name="wp")
            nc.gpsimd.dma_start(wp_all, wpt)
            bp_sb = big.tile([P, E], F32, tag="bp", name="bp")
            nc.gpsimd.dma_start(
                bp_sb, bass.AP(tensor=bp4.tensor, offset=bp4.offset,
                               ap=[[0, P]] + list(bp4.ap)))

            q_sb = [big.tile([P, T], BF16, tag=f"q{hc}", name=f"q{hc}")
                    for hc in range(2)]
            k_sb = [big.tile([P, T], BF16, tag=f"k{hc}", name=f"k{hc}")
                    for hc in range(2)]
            at_sb = [big.tile([P, T], BF16, tag=f"at{hc}", name=f"at{hc}")
                     for hc in range(2)]
            v_sb = [big.tile([P, HL, 2 * D], BF16, tag=f"v{t}", name=f"v{t}")
                    for t in range(N_TC)]

            def v_finish(t_, ps):
                nc.gpsimd.tensor_copy(v_sb[t_][:, :, 0:D], ones_sb)
                nc.vector.tensor_add(
                    v_sb[t_][:, :, D:2 * D],
                    ps.rearrange("p (h d) -> p h d", h=HL),
                    bv_sb.rearrange("p (h d) -> p h d", h=HL))

            # -------- pre-attention: q/k(tb0) both head pairs + v(t0-3) ----
            # ec-inner across 8 concurrently-open PSUM groups: every
            # xt(ec,0) chunk arrival unlocks one matmul per group.
            # pre-attention: q/k for tb0 AND tb1, both head pairs -- 8 PSUM
            # groups, ec-interleaved in DMA-arrival order.  8 matmuls
            # (1.73us) per 512KB chunk matches the ~2us delivery cadence,
            # so the PE stays saturated from the first arrival; 14 warm-up
            # matmuls before that complete the p-state ramp.  v(t0-7)
            # happens inside the first two attention calls as fillers.
            with tc.tile_pool(name="preA", bufs=8, space="PSUM") as preA:
                warm_ps = preA.tile([P, TQ], F32, tag="mm", name="mm")

                def warm(n):
                    for _ in range(n):
                        nc.tensor.matmul(warm_ps[:, 0:HDL],
                                         lhsT=warm_sb[:, 0:P], rhs=warm_sb,
                                         start=True, stop=True)

                # ~5us of warm-up: input delivery is ~300GB/s COMBINED
                # across rings, so the first xt chunk pair lands ~13us and
                # the rest every ~3.3us; 2 qk groups (3.46us) consume each
                # pair, warms cover the ramp and the residual gaps.
                warm(24)
                pss = {}
                for tb in range(2):
                    for wi in range(2):
                        for hc in range(2):
                            # 8 allocations: the last rotates onto the
                            # warm-up slot (free once the warms retire)
                            pss[(tb, wi, hc)] = preA.tile(
                                [P, TQ], F32, tag="mm", name="mm")
                for ec in range(N_EC):
                    for tb in range(2):
                        for wi, w_all in ((0, wq_all), (1, wk_all)):
                            for hc in range(2):
                                nc.tensor.matmul(
                                    pss[(tb, wi, hc)],
                                    lhsT=w_all[:, ec, hc * P:(hc + 1) * P],
                                    rhs=xts(ec, tb),
                                    start=(ec == 0), stop=(ec == N_EC - 1))
                # bias adds ride ScalarE (idle until attention): the Vector
                # queue at the preA->attention boundary otherwise backs up
                # behind these 8 adds and delays the ib0 mask/normalize
                # chain by several us
                for tb in range(2):
                    for wi, (bias_t, dst) in ((0, (bq_sb, q_sb)),
                                              (1, (bk_sb, k_sb))):
                        for hc in range(2):
                            nc.scalar.activation(
                                dst[hc][:, tb * TQ:(tb + 1) * TQ],
                                pss[(tb, wi, hc)], AF.Identity,
                                bias=bias_t[:, hc:hc + 1])

            # -------- attention (block-causal, per head pair) --------------
            # j-loop software-pipelined one step; filler closures absorb
            # the PE slack while ScalarE's exp drains.
            import contextlib
            _ph34 = contextlib.ExitStack()
            stps = _ph34.enter_context(
                tc.tile_pool(name="stps", bufs=2, space="PSUM"))
            accps = _ph34.enter_context(
                tc.tile_pool(name="accps", bufs=1, space="PSUM"))
            mmps = _ph34.enter_context(
                tc.tile_pool(name="mmps", bufs=2, space="PSUM"))

            # pre-emitted score tiles: exp work for a future attention call
            # run early (as filler) while ScalarE has slack, so the
            # exp-heavy final i-blocks don't stall the PE.  Pre-emitted pt
            # tiles use a dedicated pool so the main pipeline's rotation
            # cannot recycle them while still live (WAR deadlock).
            prep = _ph34.enter_context(tc.tile_pool(name="prep", bufs=12))
            pre = {}

            def emit_score(hp, ib, jb, pool):
                idx = jb - 4 * ib       # >= 0 on the block diagonal
                dd = idx * JB if idx >= 0 else 0
                st = stps.tile([P, 2, TQ], F32, tag="st", name="st")
                pt = pool.tile([P, 2, TQ], BF16, tag="pt", name="pt")
                for h in range(2):
                    pr = slice(h * D, (h + 1) * D)
                    nc.tensor.matmul(
                        st[:, h, dd:],
                        lhsT=q_sb[hp][pr, jb * JB:(jb + 1) * JB],
                        rhs=k_sb[hp][pr, ib * TQ + dd:(ib + 1) * TQ],
                        start=True, stop=True)
                nc.scalar.activation(pt[:, :, dd:], st[:, :, dd:],
                                     AF.Exp, scale=0.125)
                if idx >= 0:
                    for h in range(2):
                        nc.vector.tensor_mul(
                            pt[:, h, dd:], pt[:, h, dd:],
                            mask_sb[:, idx, dd:])
                return pt, dd

            def pre_score(hp, ib, jb):
                def go():
                    pre[(hp, ib, jb)] = emit_score(hp, ib, jb, prep)
                return go

            def attention(hp, ibs=None, filler=None, last=False):
                filler = list(filler) if filler else []
                for ib in (range(N_TB) if ibs is None else ibs):
                    njb = 4 * ib + 4
                    accs = [accps.tile([2 * D, TQ], F32, tag=f"acc{h}",
                                       name=f"acc{h}") for h in range(2)]

                    def s_emit(jb):
                        key = (hp, ib, jb)
                        if key in pre:
                            return pre.pop(key)
                        return emit_score(hp, ib, jb, work)

                    # scores are emitted in batches of two j-steps so the PE
                    # switches tiling mode (64x128 scores <-> 128x128
                    # everything else; each switch drains the array, ~105ns)
                    # once per batch instead of once per step.  PV(jb) reads
                    # pt (SBUF), not st, so the 2-deep st pool still
                    # suffices: st(jb+2) lands in the slot exp(jb) freed.
                    pend = {0: s_emit(0), 1: s_emit(1)}
                    for jp in range(0, njb, 2):
                        # group 0: filler BEFORE the next score batch --
                        # s_emit(2) needs the st slot exp(0) frees, and
                        # exp(0) only starts once s(0)/s(1) finish, so the
                        # filler covers that latency; later groups keep the
                        # scores up front to minimize tiling-mode switches.
                        if jp == 0 and filler:
                            filler.pop(0)()
                        for jn in (jp + 2, jp + 3):
                            if jn < njb:
                                pend[jn] = s_emit(jn)
                        if jp > 0 and filler:
                            filler.pop(0)()
                        for jb in (jp, jp + 1):
                            pt, dd = pend.pop(jb)
                            for h in range(2):
                                nc.tensor.matmul(
                                    accs[h][:, dd:],
                                    lhsT=v_sb[jb][:, 2 * hp + h, :],
                                    rhs=pt[:, h, dd:],
                                    start=(jb == 0), stop=(jb == njb - 1))
                    if not last:
                        for h in range(2):
                            rec64 = work.tile([D, TQ], F32, tag="rec64",
                                              name="rec64")
                            nc.vector.reciprocal_approx_fast(rec64,
                                                             accs[h][0:D, :])
                            nc.vector.tensor_mul(
                                at_sb[hp][h * D:(h + 1) * D,
                                          ib * TQ:(ib + 1) * TQ],
                                accs[h][D:2 * D, :], rec64)
                    else:
                        # final i-block: normalize in 128-col chunks so the
                        # trailing output projections start ~0.8us after the
                        # last PV instead of waiting the full [64,512] pass
                        for cc in range(4):
                            sl = slice(cc * P, (cc + 1) * P)
                            for h in range(2):
                                rec = work.tile([D, P], F32, tag="recc",
                                                name="recc")
                                nc.vector.reciprocal_approx_fast(
                                    rec, accs[h][0:D, sl])
                                nc.vector.tensor_mul(
                                    at_sb[hp][h * D:(h + 1) * D,
                                              ib * TQ + cc * P:
                                              ib * TQ + (cc + 1) * P],
                                    accs[h][D:2 * D, sl], rec)
                while filler:
                    filler.pop(0)()

            # ---- filler closures (emitted between attention j-steps) ------
            def qk2_group(hc, tb, wi):
                w_all = (wq_all, wk_all)[wi]
                bias_t = (bq_sb, bk_sb)[wi]
                dst = (q_sb, k_sb)[wi]

                def go():
                    ps = mmps.tile([P, TQ], F32, tag="mm", name="mm")
                    for ec in range(N_EC):
                        nc.tensor.matmul(
                            ps,
                            lhsT=w_all[:, ec, hc * P:(hc + 1) * P],
                            rhs=xts(ec, tb),
                            start=(ec == 0), stop=(ec == N_EC - 1))
                    nc.vector.tensor_scalar_add(
                        dst[hc][:, tb * TQ:(tb + 1) * TQ], ps,
                        bias_t[:, hc:hc + 1])
                return go

            def v_group(t_):
                def go():
                    ps = mmps.tile([P, HDL], F32, tag="mm", name="mm")
                    for ec in range(N_EC):
                        nc.tensor.matmul(
                            ps,
                            lhsT=xts(ec, t_ // 4)[
                                :, (t_ % 4) * P:(t_ % 4 + 1) * P],
                            rhs=wv_all[:, ec, :],
                            start=(ec == 0), stop=(ec == N_EC - 1))
                    v_finish(t_, ps)
                return go

            def proj_t(t_):
                def go():
                    ot = outp.tile([P, E], BF16, tag="ot", name="ot")
                    for eb in range(2):
                        ps = mmps.tile([P, TQ], F32, tag="mm", name="mm")
                        for hc in range(2):
                            nc.tensor.matmul(
                                ps,
                                lhsT=at_sb[hc][:, t_ * P:(t_ + 1) * P],
                                rhs=wp_all[:, hc, eb * TQ:(eb + 1) * TQ],
                                start=(hc == 0), stop=(hc == 1))
                        nc.vector.tensor_add(
                            ot[:, eb * TQ:(eb + 1) * TQ], ps,
                            bp_sb[:, eb * TQ:(eb + 1) * TQ])
                    # the last four stores alternate rings so they drain in
                    # parallel instead of serializing ~1.1us each on sync
                    eng = nc.scalar if t_ >= 12 and t_ % 2 else nc.sync
                    eng.dma_start(out[t_ * P:(t_ + 1) * P, :], ot)
                return go

            # interleave the two head pairs at i-block granularity; fillers
            # deliver each (hp, ib)'s q/k/v deps at least one call ahead and
            # drain proj as soon as both head pairs finished an i-block.
            def bundle(*fs):
                def go():
                    for f in fs:
                        f()
                return go

            # q/k for tb0+tb1 exist from preA; v(t0-7) rides the first two
            # calls as bundled fillers (each bundle before the PVs that
            # read it); tb2/tb3 q/k and the remaining v/proj interleave
            # later calls; pre_score cascades ib3 exp work forward so the
            # exp-heavy final calls stay PE-bound.
            attention(0, ibs=[0],
                      filler=[bundle(v_group(0), v_group(1)),
                              bundle(v_group(2), v_group(3))])
            attention(1, ibs=[0],
                      filler=[bundle(v_group(4), v_group(5)),
                              bundle(v_group(6), v_group(7))])
            attention(0, ibs=[1],
                      filler=[qk2_group(0, 2, 0), qk2_group(0, 2, 1),
                              v_group(8), v_group(9)])
            attention(1, ibs=[1],
                      filler=[qk2_group(1, 2, 0), qk2_group(1, 2, 1),
                              v_group(10), v_group(11)])
            attention(0, ibs=[2],
                      filler=[qk2_group(0, 3, 0), qk2_group(0, 3, 1),
                              v_group(12), v_group(13),
                              proj_t(0), proj_t(1),
                              pre_score(0, 3, 0), pre_score(0, 3, 1),
                              pre_score(0, 3, 2), pre_score(0, 3, 3)])
            attention(1, ibs=[2],
                      filler=[qk2_group(1, 3, 0), qk2_group(1, 3, 1),
                              v_group(14), v_group(15),
                              proj_t(2), proj_t(3),
                              pre_score(1, 3, 0), pre_score(1, 3, 1)])
            attention(0, ibs=[3],
                      filler=[proj_t(4), proj_t(5), proj_t(6), proj_t(7),
                              proj_t(8),
                              pre_score(1, 3, 2), pre_score(1, 3, 3),
                              pre_score(1, 3, 4), pre_score(1, 3, 5),
                              pre_score(1, 3, 6), pre_score(1, 3, 7),
                              pre_score(1, 3, 8), pre_score(1, 3, 9)])
            attention(1, ibs=[3],
                      filler=[proj_t(9), proj_t(10), proj_t(11)])
            for t_ in range(12, 16):
                proj_t(t_)()
            _ph34.close()

    nc.compile()
    return nc


def _make_mask():
    jj = np.arange(JB)[:, None]
    ii = np.arange(TQ)[None, :]
    m = np.zeros((GROUPS, JB, TQ), dtype=np.float32)
    for d in range(GROUPS):
        m[d] = (jj + d * JB <= ii).astype(np.float32)
    return m.astype(ml_dtypes.bfloat16)


_NC = None


def _get_nc():
    global _NC
    if _NC is None:
        _NC = _build_nc()
    return _NC


def _warr(w):
    """W slice [HDL, E] -> SBUF layout [P, N_EC, HDL]: element (p, c, f) =
    W.T[c*P + p, f]."""
    return np.ascontiguousarray(
        w.T.reshape(N_EC, P, HDL).transpose(1, 0, 2)).astype(ml_dtypes.bfloat16)


def kernel(x, Wq, bq, Wk, bk, Wv, bv, Wp, bp, **_run_kwargs):
    x = np.asarray(x, dtype=np.float32)
    Wq = np.asarray(Wq, dtype=np.float32)
    Wk = np.asarray(Wk, dtype=np.float32)
    Wv = np.asarray(Wv, dtype=np.float32)
    Wp = np.asarray(Wp, dtype=np.float32)
    bq = np.asarray(bq, dtype=np.float32)
    bk = np.asarray(bk, dtype=np.float32)
    bv = np.asarray(bv, dtype=np.float32)
    bp = np.asarray(bp, dtype=np.float32)

    mask = _make_mask()
    bp4 = (bp / GROUPS).astype(np.float32)

    in_maps = []
    for c in range(NCORES):
        b, hg = divmod(c, GROUPS)
        hsl = slice(HDL * hg, HDL * (hg + 1))
        in_maps.append({
            "xt": np.ascontiguousarray(x[b].T).astype(ml_dtypes.bfloat16),
            "wqt": _warr(Wq[hsl]),
            "wkt": _warr(Wk[hsl]),
            "wvt": _warr(Wv[hsl]),
            "wpt": np.ascontiguousarray(
                Wp[:, hsl].T.reshape(2, P, E).transpose(1, 0, 2)
            ).astype(ml_dtypes.bfloat16),
            "bqv": np.ascontiguousarray(bq[hsl]),
            "bkv": np.ascontiguousarray(bk[hsl]),
            "bvv": np.ascontiguousarray(bv[hsl]),
            "bp4": bp4,
            "mask": mask,
            "onesv": np.ones(HDL, dtype=ml_dtypes.bfloat16),
        })

    nc = _get_nc()
    try:
        res = run_bass_kernel_spmd(nc, in_maps, core_ids=list(range(NCORES)),
                                   **_run_kwargs)
    except Exception:
        # transient device hiccups (e.g. NRT_EXEC_UNIT_UNRECOVERABLE) have
        # been observed to clear on retry
        import time
        time.sleep(2.0)
        res = run_bass_kernel_spmd(nc, in_maps, core_ids=list(range(NCORES)),
                                   **_run_kwargs)
    outs = [r["out"].astype(np.float32) for r in res.results]
    y = np.stack([
        outs[0] + outs[1] + outs[2] + outs[3],
        outs[4] + outs[5] + outs[6] + outs[7],
    ]).astype(np.float32)
    if _run_kwargs:
        return y, res
    return y


# revision 35
# speedup vs baseline: 1.0219x; 1.0121x over previous
"""Causal attention (B=2, T=2048, E=1024, H=16, D=64) on 8 TRN2 NeuronCores.

Sharding: core c handles batch b = c//4 and local head group hg = c%4
(4 heads, 256 head-dims).  Data parallel over batch, tensor parallel over
heads; the output projection is row-parallel, so each core returns a
partial [T, E] output and the host sums the 4 partials per batch (bias
is pre-divided by 4 and added on-device).

Device plan (per core, all-bf16 matmuls with fp32 PSUM accumulation):
  xt  = x[b].T                   [E, T]  (host-transposed; e on partitions)
  wqt/wkt/wvt = W[h].T           pre-tiled [P, 8, 256] for SBUF layout
  wpt = Wp[:, h].T               pre-tiled [P, 2, 1024]

Schedule: TRN2's PE p-state ramps (0.65->1.2->2.4 GHz over ~3us of
CONTINUOUS work; any idle gap resets it), so the whole kernel is built
as one unbroken PE instruction stream:
  - xt is DMAed in 32 [128,512] chunks in tb-major (consumption) order
    on the sync ring; wq/wk (in halves) + wv stream on the scalar ring.
    First matmul starts as soon as wq-half0 + xt(ec0,tb0) land.
  - pre-attention: q/k for token-block tb0 (both head pairs) + v(t0-3)
    in an 8-bank PSUM pool, ec-inner so matmuls chase chunk arrivals.
  - attention(hp, ib) runs as soon as its q/k/v deps exist; ALL other
    work (q/k tb1-3, v t4-15, output projection) is emitted as ~1-2us
    filler closures between attention pipeline steps, sized so the PE
    never idles while ScalarE's exp drains (j-loop software-pipelined
    one step: scores/exp for jb+1 are emitted before the PV matmuls of
    jb).
  - scores st[j, i] = q_j . k_i with 2-head row-packing (two K=64
    matmuls in distinct PE row groups), exp on ScalarE (scale=1/8; no
    max subtraction -- scores are ~N(0,1) so exp cannot overflow),
    causal mask multiply only on block-diagonal tiles, PV accumulation
    over j in PSUM with 64 ones-columns prepended to v (softmax
    denominator comes out of the PV matmul on partitions 0:63), then
    approx-reciprocal + multiply for the normalization.
  - v ones-columns are written by GpSimd (otherwise idle) to keep
    VectorE off the critical path.
"""

import ml_dtypes
import numpy as np

import concourse.bass as bass
import concourse.tile as tile
from concourse import bacc, mybir
from concourse.bass_utils import run_bass_kernel_spmd

B, T, E = 2, 2048, 1024
H, D = 16, 64
NCORES = 8
GROUPS = 4              # cores per batch (tensor parallel over heads)
HL = H // GROUPS        # 4 local heads per core
HDL = HL * D            # 256 local head dims
P = 128
TQ = 512                # i-block (free dim of score tiles)
JB = 128                # j-block (partition dim of score tiles)
N_TB = T // TQ          # 4
N_EC = E // P           # 8
N_TC = T // P           # 16

F32 = mybir.dt.float32
BF16 = mybir.dt.bfloat16
AF = mybir.ActivationFunctionType


def _build_nc():
    nc = bacc.Bacc("TRN2", target_bir_lowering=False, debug=False)
    xt = nc.dram_tensor("xt", [E, T], BF16, kind="ExternalInput").ap()
    wqt = nc.dram_tensor("wqt", [P, N_EC, HDL], BF16, kind="ExternalInput").ap()
    wkt = nc.dram_tensor("wkt", [P, N_EC, HDL], BF16, kind="ExternalInput").ap()
    wvt = nc.dram_tensor("wvt", [P, N_EC, HDL], BF16, kind="ExternalInput").ap()
    wpt = nc.dram_tensor("wpt", [P, 2, E], BF16, kind="ExternalInput").ap()
    bqv = nc.dram_tensor("bqv", [HDL], F32, kind="ExternalInput").ap()
    bkv = nc.dram_tensor("bkv", [HDL], F32, kind="ExternalInput").ap()
    bvv = nc.dram_tensor("bvv", [HDL], F32, kind="ExternalInput").ap()
    bp4 = nc.dram_tensor("bp4", [E], F32, kind="ExternalInput").ap()
    maskd = nc.dram_tensor("mask", [GROUPS, JB, TQ], BF16,
                           kind="ExternalInput").ap()
    onesv = nc.dram_tensor("onesv", [HDL], BF16, kind="ExternalInput").ap()
    out = nc.dram_tensor("out", [T, E], BF16, kind="ExternalOutput").ap()

    with tile.TileContext(nc) as tc:
        with (
            tc.tile_pool(name="big", bufs=1) as big,
            tc.tile_pool(name="work", bufs=5) as work,
            tc.tile_pool(name="outp", bufs=3) as outp,
        ):
            # ---------------- input loads ---------------------------------
            # DMA rings move contiguous 512KB descriptors at ~330GB/s but
            # strided gathers collapse to ~35GB/s, so xt ships as 8 full-T
            # contiguous e-chunks alternating between the two HWDGE rings;
            # weights lead the scalar ring, wv rides the sync ring between
            # xt chunks.  Expected arrivals (ring start ~6.7us, 1.55us per
            # 512KB): sync e0 8.3, e2 9.9, wv 11.4, e4 13, e6 14.6;
            # scalar wq 8.3, wk 9.9, e1 11.4, e3 13, e5 14.6, e7 16.1.
            xt_sb = [big.tile([P, T], BF16, tag=f"xt{ec}", name=f"xt{ec}")
                     for ec in range(N_EC)]
            wq_all = big.tile([P, N_EC, HDL], BF16, tag="wq", name="wq")
            wk_all = big.tile([P, N_EC, HDL], BF16, tag="wk", name="wk")
            wv_all = big.tile([P, N_EC, HDL], BF16, tag="wv", name="wv")

            def xt_dma(eng, ec):
                eng.dma_start(xt_sb[ec], xt[ec * P:(ec + 1) * P, :])

            nc.sync.dma_start(wq_all, wqt)
            nc.scalar.dma_start(wk_all, wkt)
            for ec in range(0, N_EC, 2):
                xt_dma(nc.sync, ec)
                xt_dma(nc.scalar, ec + 1)

            def xts(ec, tb):
                return xt_sb[ec][:, tb * TQ:(tb + 1) * TQ]
            # PE warm-up scratch: matmuls on this (memset) tile run while
            # the first input DMAs are in flight, so the PE p-state ramp
            # (0.65->2.4GHz over ~3us of continuous work) completes before
            # real data arrives instead of slowing the first real matmuls.
            warm_sb = big.tile([P, HDL], BF16, tag="warm", name="warm")
            nc.gpsimd.memset(warm_sb, 0.0)
            # gpsimd (SWDGE): small / late-needed tensors
            bv_sb = big.tile([P, HDL], F32, tag="bv", name="bv")
            nc.gpsimd.dma_start(
                bv_sb, bass.AP(tensor=bvv.tensor, offset=bvv.offset,
                               ap=[[0, P]] + list(bvv.ap)))
            bq_sb = big.tile([P, 2], F32, tag="bq", name="bq")
            nc.gpsimd.dma_start(bq_sb, bqv.rearrange("(c p) -> p c", p=P))
            bk_sb = big.tile([P, 2], F32, tag="bk", name="bk")
            nc.gpsimd.dma_start(bk_sb, bkv.rearrange("(c p) -> p c", p=P))
            ones_sb = big.tile([P, HL, D], BF16, tag="ones", name="ones")
            ones_r = onesv.rearrange("(h d) -> h d", h=HL)
            nc.gpsimd.dma_start(
                ones_sb, bass.AP(tensor=onesv.tensor, offset=onesv.offset,
                                 ap=[[0, P]] + list(ones_r.ap)))
            mask_sb = big.tile([P, GROUPS, TQ], BF16, tag="mask", name="mask")
            nc.gpsimd.dma_start(mask_sb, maskd.rearrange("d p f -> p d f"))
            # wv last on the sync ring: needed only by the v fillers in the
            # first attention call, after all xt chunks (SWDGE is too slow
            # for it -- measured ~3us stall)
            nc.sync.dma_start(wv_all, wvt)
            wp_all = big.tile([P, 2, E], BF16, tag="wp", name="wp")
            nc.gpsimd.dma_start(wp_all, wpt)
            bp_sb = big.tile([P, E], F32, tag="bp", name="bp")
            nc.gpsimd.dma_start(
                bp_sb, bass.AP(tensor=bp4.tensor, offset=bp4.offset,
                               ap=[[0, P]] + list(bp4.ap)))

            q_sb = [big.tile([P, T], BF16, tag=f"q{hc}", name=f"q{hc}")
                    for hc in range(2)]
            k_sb = [big.tile([P, T], BF16, tag=f"k{hc}", name=f"k{hc}")
                    for hc in range(2)]
            at_sb = [big.tile([P, T], BF16, tag=f"at{hc}", name=f"at{hc}")
                     for hc in range(2)]
            v_sb = [big.tile([P, HL, 2 * D], BF16, tag=f"v{t}", name=f"v{t}")
                    for t in range(N_TC)]

            def v_finish(t_, ps):
                nc.gpsimd.tensor_copy(v_sb[t_][:, :, 0:D], ones_sb)
                nc.vector.tensor_add(
                    v_sb[t_][:, :, D:2 * D],
                    ps.rearrange("p (h d) -> p h d", h=HL),
                    bv_sb.rearrange("p (h d) -> p h d", h=HL))

            # -------- pre-attention: q/k(tb0) both head pairs + v(t0-3) ----
            # ec-inner across 8 concurrently-open PSUM groups: every
            # xt(ec,0) chunk arrival unlocks one matmul per group.
            # pre-attention: q/k for tb0 AND tb1, both head pairs -- 8 PSUM
            # groups, ec-interleaved in DMA-arrival order.  8 matmuls
            # (1.73us) per 512KB chunk matches the ~2us delivery cadence,
            # so the PE stays saturated from the first arrival; 14 warm-up
            # matmuls before that complete the p-state ramp.  v(t0-7)
            # happens inside the first two attention calls as fillers.
            with tc.tile_pool(name="preA", bufs=8, space="PSUM") as preA:
                warm_ps = preA.tile([P, TQ], F32, tag="mm", name="mm")

                def warm(n):
                    for _ in range(n):
                        nc.tensor.matmul(warm_ps[:, 0:HDL],
                                         lhsT=warm_sb[:, 0:P], rhs=warm_sb,
                                         start=True, stop=True)

                # ~6.5us of warm-up: input delivery is ~300GB/s COMBINED
                # across rings, so the first xt chunk pair lands ~13us and
                # the rest every ~3.3us; 2 qk groups (3.46us) consume each
                # pair, warms cover the ramp and the residual gaps.
                warm(28)
                pss = {}
                for tb in range(2):
                    for wi in range(2):
                        for hc in range(2):
                            # 8 allocations: the last rotates onto the
                            # warm-up slot (free once the warms retire)
                            pss[(tb, wi, hc)] = preA.tile(
                                [P, TQ], F32, tag="mm", name="mm")
                for ec in range(N_EC):
                    for tb in range(2):
                        for wi, w_all in ((0, wq_all), (1, wk_all)):
                            for hc in range(2):
                                nc.tensor.matmul(
                                    pss[(tb, wi, hc)],
                                    lhsT=w_all[:, ec, hc * P:(hc + 1) * P],
                                    rhs=xts(ec, tb),
                                    start=(ec == 0), stop=(ec == N_EC - 1))
                # bias adds ride ScalarE (idle until attention): the Vector
                # queue at the preA->attention boundary otherwise backs up
                # behind these 8 adds and delays the ib0 mask/normalize
                # chain by several us
                for tb in range(2):
                    for wi, (bias_t, dst) in ((0, (bq_sb, q_sb)),
                                              (1, (bk_sb, k_sb))):
                        for hc in range(2):
                            nc.scalar.activation(
                                dst[hc][:, tb * TQ:(tb + 1) * TQ],
                                pss[(tb, wi, hc)], AF.Identity,
                                bias=bias_t[:, hc:hc + 1])

            # -------- attention (block-causal, per head pair) --------------
            # j-loop software-pipelined one step; filler closures absorb
            # the PE slack while ScalarE's exp drains.
            import contextlib
            _ph34 = contextlib.ExitStack()
            stps = _ph34.enter_context(
                tc.tile_pool(name="stps", bufs=2, space="PSUM"))
            accps = _ph34.enter_context(
                tc.tile_pool(name="accps", bufs=1, space="PSUM"))
            mmps = _ph34.enter_context(
                tc.tile_pool(name="mmps", bufs=2, space="PSUM"))

            # pre-emitted score tiles: exp work for a future attention call
            # run early (as filler) while ScalarE has slack, so the
            # exp-heavy final i-blocks don't stall the PE.  Pre-emitted pt
            # tiles use a dedicated pool so the main pipeline's rotation
            # cannot recycle them while still live (WAR deadlock).
            prep = _ph34.enter_context(tc.tile_pool(name="prep", bufs=16))
            pre = {}

            def emit_score(hp, ib, jb, pool):
                idx = jb - 4 * ib       # >= 0 on the block diagonal
                dd = idx * JB if idx >= 0 else 0
                st = stps.tile([P, 2, TQ], F32, tag="st", name="st")
                pt = pool.tile([P, 2, TQ], BF16, tag="pt", name="pt")
                for h in range(2):
                    pr = slice(h * D, (h + 1) * D)
                    nc.tensor.matmul(
                        st[:, h, dd:],
                        lhsT=q_sb[hp][pr, jb * JB:(jb + 1) * JB],
                        rhs=k_sb[hp][pr, ib * TQ + dd:(ib + 1) * TQ],
                        start=True, stop=True)
                nc.scalar.activation(pt[:, :, dd:], st[:, :, dd:],
                                     AF.Exp, scale=0.125)
                if idx >= 0:
                    for h in range(2):
                        nc.vector.tensor_mul(
                            pt[:, h, dd:], pt[:, h, dd:],
                            mask_sb[:, idx, dd:])
                return pt, dd

            def pre_score(hp, ib, jb):
                def go():
                    pre[(hp, ib, jb)] = emit_score(hp, ib, jb, prep)
                return go

            def attention(hp, ibs=None, filler=None, last=False):
                filler = list(filler) if filler else []
                for ib in (range(N_TB) if ibs is None else ibs):
                    njb = 4 * ib + 4
                    accs = [accps.tile([2 * D, TQ], F32, tag=f"acc{h}",
                                       name=f"acc{h}") for h in range(2)]

                    def s_emit(jb):
                        key = (hp, ib, jb)
                        if key in pre:
                            return pre.pop(key)
                        return emit_score(hp, ib, jb, work)

                    # scores are emitted in batches of two j-steps so the PE
                    # switches tiling mode (64x128 scores <-> 128x128
                    # everything else; each switch drains the array, ~105ns)
                    # once per batch instead of once per step.  PV(jb) reads
                    # pt (SBUF), not st, so the 2-deep st pool still
                    # suffices: st(jb+2) lands in the slot exp(jb) freed.
                    pend = {0: s_emit(0), 1: s_emit(1)}
                    for jp in range(0, njb, 2):
                        # group 0: filler BEFORE the next score batch --
                        # s_emit(2) needs the st slot exp(0) frees, and
                        # exp(0) only starts once s(0)/s(1) finish, so the
                        # filler covers that latency; later groups keep the
                        # scores up front to minimize tiling-mode switches.
                        if jp == 0 and filler:
                            filler.pop(0)()
                        for jn in (jp + 2, jp + 3):
                            if jn < njb:
                                pend[jn] = s_emit(jn)
                        if jp > 0 and filler:
                            filler.pop(0)()
                        for jb in (jp, jp + 1):
                            pt, dd = pend.pop(jb)
                            for h in range(2):
                                nc.tensor.matmul(
                                    accs[h][:, dd:],
                                    lhsT=v_sb[jb][:, 2 * hp + h, :],
                                    rhs=pt[:, h, dd:],
                                    start=(jb == 0), stop=(jb == njb - 1))
                    if not last:
                        for h in range(2):
                            rec64 = work.tile([D, TQ], F32, tag="rec64",
                                              name="rec64")
                            nc.vector.reciprocal_approx_fast(rec64,
                                                             accs[h][0:D, :])
                            nc.vector.tensor_mul(
                                at_sb[hp][h * D:(h + 1) * D,
                                          ib * TQ:(ib + 1) * TQ],
                                accs[h][D:2 * D, :], rec64)
                    else:
                        # final i-block: normalize in 128-col chunks so the
                        # trailing output projections start ~0.8us after the
                        # last PV instead of waiting the full [64,512] pass
                        for cc in range(4):
                            sl = slice(cc * P, (cc + 1) * P)
                            for h in range(2):
                                rec = work.tile([D, P], F32, tag="recc",
                                                name="recc")
                                nc.vector.reciprocal_approx_fast(
                                    rec, accs[h][0:D, sl])
                                nc.vector.tensor_mul(
                                    at_sb[hp][h * D:(h + 1) * D,
                                              ib * TQ + cc * P:
                                              ib * TQ + (cc + 1) * P],
                                    accs[h][D:2 * D, sl], rec)
                while filler:
                    filler.pop(0)()

            # ---- filler closures (emitted between attention j-steps) ------
            def qk2_group(hc, tb, wi):
                w_all = (wq_all, wk_all)[wi]
                bias_t = (bq_sb, bk_sb)[wi]
                dst = (q_sb, k_sb)[wi]

                def go():
                    ps = mmps.tile([P, TQ], F32, tag="mm", name="mm")
                    for ec in range(N_EC):
                        nc.tensor.matmul(
                            ps,
                            lhsT=w_all[:, ec, hc * P:(hc + 1) * P],
                            rhs=xts(ec, tb),
                            start=(ec == 0), stop=(ec == N_EC - 1))
                    nc.vector.tensor_scalar_add(
                        dst[hc][:, tb * TQ:(tb + 1) * TQ], ps,
                        bias_t[:, hc:hc + 1])
                return go

            def v_group(t_):
                def go():
                    ps = mmps.tile([P, HDL], F32, tag="mm", name="mm")
                    for ec in range(N_EC):
                        nc.tensor.matmul(
                            ps,
                            lhsT=xts(ec, t_ // 4)[
                                :, (t_ % 4) * P:(t_ % 4 + 1) * P],
                            rhs=wv_all[:, ec, :],
                            start=(ec == 0), stop=(ec == N_EC - 1))
                    v_finish(t_, ps)
                return go

            def proj_t(t_):
                def go():
                    ot = outp.tile([P, E], BF16, tag="ot", name="ot")
                    for eb in range(2):
                        ps = mmps.tile([P, TQ], F32, tag="mm", name="mm")
                        for hc in range(2):
                            nc.tensor.matmul(
                                ps,
                                lhsT=at_sb[hc][:, t_ * P:(t_ + 1) * P],
                                rhs=wp_all[:, hc, eb * TQ:(eb + 1) * TQ],
                                start=(hc == 0), stop=(hc == 1))
                        nc.vector.tensor_add(
                            ot[:, eb * TQ:(eb + 1) * TQ], ps,
                            bp_sb[:, eb * TQ:(eb + 1) * TQ])
                    # the last four stores alternate rings so they drain in
                    # parallel instead of serializing ~1.1us each on sync
                    eng = nc.scalar if t_ >= 12 and t_ % 2 else nc.sync
                    eng.dma_start(out[t_ * P:(t_ + 1) * P, :], ot)
                return go

            # interleave the two head pairs at i-block granularity; fillers
            # deliver each (hp, ib)'s q/k/v deps at least one call ahead and
            # drain proj as soon as both head pairs finished an i-block.
            def bundle(*fs):
                def go():
                    for f in fs:
                        f()
                return go

            # q/k for tb0+tb1 exist from preA; v(t0-7) rides the first two
            # calls as bundled fillers (each bundle before the PVs that
            # read it); tb2/tb3 q/k and the remaining v/proj interleave
            # later calls; pre_score cascades ib3 exp work forward so the
            # exp-heavy final calls stay PE-bound.
            attention(0, ibs=[0],
                      filler=[bundle(v_group(0), v_group(1)),
                              bundle(v_group(2), v_group(3))])
            attention(1, ibs=[0],
                      filler=[bundle(v_group(4), v_group(5)),
                              bundle(v_group(6), v_group(7))])
            attention(0, ibs=[1],
                      filler=[qk2_group(0, 2, 0), qk2_group(0, 2, 1),
                              v_group(8), v_group(9)])
            attention(1, ibs=[1],
                      filler=[qk2_group(1, 2, 0), qk2_group(1, 2, 1),
                              v_group(10), v_group(11)])
            # pre_scores are bundled with proj/v fillers (never drained in a
            # bulk run: the 2-deep st pool would pace a bulk at exp rate),
            # cascading ib3's exp load into earlier PE-bound windows.
            attention(0, ibs=[2],
                      filler=[qk2_group(0, 3, 0), qk2_group(0, 3, 1),
                              bundle(v_group(12), pre_score(0, 3, 0)),
                              bundle(v_group(13), pre_score(0, 3, 1)),
                              bundle(proj_t(0), pre_score(0, 3, 2)),
                              bundle(proj_t(1), pre_score(0, 3, 3))])
            attention(1, ibs=[2],
                      filler=[qk2_group(1, 3, 0), qk2_group(1, 3, 1),
                              bundle(v_group(14), pre_score(1, 3, 0)),
                              bundle(v_group(15), pre_score(1, 3, 1)),
                              bundle(proj_t(2), pre_score(0, 3, 4)),
                              bundle(proj_t(3), pre_score(0, 3, 5),
                                     pre_score(0, 3, 6))])
            attention(0, ibs=[3],
                      filler=[bundle(proj_t(4), pre_score(1, 3, 2)),
                              bundle(proj_t(5), pre_score(1, 3, 3)),
                              bundle(proj_t(6), pre_score(1, 3, 4)),
                              bundle(proj_t(7), pre_score(1, 3, 5)),
                              bundle(proj_t(8), pre_score(1, 3, 6)),
                              bundle(pre_score(1, 3, 7))])
            attention(1, ibs=[3],
                      filler=[proj_t(9), proj_t(10), proj_t(11)])
            for t_ in range(12, 16):
                proj_t(t_)()
            _ph34.close()

    nc.compile()
    return nc


def _make_mask():
    jj = np.arange(JB)[:, None]
    ii = np.arange(TQ)[None, :]
    m = np.zeros((GROUPS, JB, TQ), dtype=np.float32)
    for d in range(GROUPS):
        m[d] = (jj + d * JB <= ii).astype(np.float32)
    return m.astype(ml_dtypes.bfloat16)


_NC = None


def _get_nc():
    global _NC
    if _NC is None:
        _NC = _build_nc()
    return _NC


def _warr(w):
    """W slice [HDL, E] -> SBUF layout [P, N_EC, HDL]: element (p, c, f) =
    W.T[c*P + p, f]."""
    return np.ascontiguousarray(
        w.T.reshape(N_EC, P, HDL).transpose(1, 0, 2)).astype(ml_dtypes.bfloat16)


def kernel(x, Wq, bq, Wk, bk, Wv, bv, Wp, bp, **_run_kwargs):
    x = np.asarray(x, dtype=np.float32)
    Wq = np.asarray(Wq, dtype=np.float32)
    Wk = np.asarray(Wk, dtype=np.float32)
    Wv = np.asarray(Wv, dtype=np.float32)
    Wp = np.asarray(Wp, dtype=np.float32)
    bq = np.asarray(bq, dtype=np.float32)
    bk = np.asarray(bk, dtype=np.float32)
    bv = np.asarray(bv, dtype=np.float32)
    bp = np.asarray(bp, dtype=np.float32)

    mask = _make_mask()
    bp4 = (bp / GROUPS).astype(np.float32)

    in_maps = []
    for c in range(NCORES):
        b, hg = divmod(c, GROUPS)
        hsl = slice(HDL * hg, HDL * (hg + 1))
        in_maps.append({
            "xt": np.ascontiguousarray(x[b].T).astype(ml_dtypes.bfloat16),
            "wqt": _warr(Wq[hsl]),
            "wkt": _warr(Wk[hsl]),
            "wvt": _warr(Wv[hsl]),
            "wpt": np.ascontiguousarray(
                Wp[:, hsl].T.reshape(2, P, E).transpose(1, 0, 2)
            ).astype(ml_dtypes.bfloat16),
            "bqv": np.ascontiguousarray(bq[hsl]),
            "bkv": np.ascontiguousarray(bk[hsl]),
            "bvv": np.ascontiguousarray(bv[hsl]),
            "bp4": bp4,
            "mask": mask,
            "onesv": np.ones(HDL, dtype=ml_dtypes.bfloat16),
        })

    nc = _get_nc()
    try:
        res = run_bass_kernel_spmd(nc, in_maps, core_ids=list(range(NCORES)),
                                   **_run_kwargs)
    except Exception:
        # transient device hiccups (e.g. NRT_EXEC_UNIT_UNRECOVERABLE) have
        # been observed to clear on retry
        import time
        time.sleep(2.0)
        res = run_bass_kernel_spmd(nc, in_maps, core_ids=list(range(NCORES)),
                                   **_run_kwargs)
    outs = [r["out"].astype(np.float32) for r in res.results]
    y = np.stack([
        outs[0] + outs[1] + outs[2] + outs[3],
        outs[4] + outs[5] + outs[6] + outs[7],
    ]).astype(np.float32)
    if _run_kwargs:
        return y, res
    return y
